# revision 42
# baseline (speedup 1.0000x reference)
"""Trainium2 Bass kernel for EventDiffusion GNN (GCNConv + GATConv, 2 layers).

Sharding: nodes partitioned into 8 contiguous ranges (one per NeuronCore);
each core aggregates messages for its 1280 destination nodes.  Layer-1
hidden states are exchanged with one AllGather (bf16) so every core can
build the full layer-2 feature table locally.

Aggregation strategy (both layers): dense block-push matmuls.  For every
(dst-block b, src-block g) pair a [128 src-slot x 128 dst-slot] count
matrix is streamed from HBM (bf16, contiguous) and used as the stationary
matmul operand against the SBUF-resident feature-table block:
    psum[b] += mask[b,g]^T @ table[g]            (80 matmuls per block)
No dma_gather anywhere (gather descriptor emission was the old bottleneck).

GCN normalization is folded node-wise: table1 rows are pre-scaled by
dinv[src] and the psum is post-scaled by dinv[dst], so layer 1 uses the
raw count mask directly.

GAT attention: alpha[s,d] = exp(leakyrelu(l1[s]+l2[d])) un-normalized --
the softmax shift is unnecessary because the final division by the
aggregated denominator makes the result scale-invariant per destination.
The attention matrix for a dst block is built block-wise on the Vector
engine from the rank-1 structure z[s,j] = l1[s] + l2b[j]:
    z (fp16) -> leakyrelu (1 fused op) -> exp (Scalar engine) -> * mask
then used as the push-matmul stationary operand.  The denominator is
accumulated through an all-ones column in the feature table.
"""

import numpy as np
import ml_dtypes

import concourse.bass as bass
import concourse.bacc as bacc
import concourse.mybir as mybir
import concourse.tile as tile
from concourse.bass_utils import run_bass_kernel_spmd

FP32 = mybir.dt.float32
BF16 = mybir.dt.bfloat16
FP16 = mybir.dt.float16

N_CORES = 8
D = 256
NPAD = 10240            # padded node count (80 blocks of 128)
PER = NPAD // N_CORES   # 1280 nodes per core
NBLK = PER // 128       # 10 dst blocks per core
NGB = NPAD // 128       # 80 src blocks (global)
HALF_G = NGB // 2       # src blocks per build chunk
CW = HALF_G * 128       # 5120 free-dim elems per chunk
TW = 257                # table2 width: 256 feats | 256: ones (denominator)
                        # (in the 2A psum, col 256 instead carries l1 = xw2@v1)

# src-block slot permutation: sections aligned with the three AllGather
# slices over dst sub-blocks (0-2, 3-6, 7-9); global block 79 lands in slot 79
AG_BLOCKS = [(0, 3), (3, 7), (7, 10)]
SEC_SLOTS = [24, 32, 24]  # slots per section (8 ranks x sub-blocks)
GLIST = [g for b0, b1 in AG_BLOCKS for g in range(NGB) if b0 <= g % NBLK < b1]
POS = {g: i for i, g in enumerate(GLIST)}

BF = ml_dtypes.bfloat16


def _bf16(a):
    return np.ascontiguousarray(np.asarray(a, np.float32).astype(BF))


# ----------------------------------------------------------------------------
# host-side preprocessing (graph metadata -> dense block count-masks)
# ----------------------------------------------------------------------------

def _prep(event_emb, edge_index, W1, b1, W2, att_src, att_dst, b2):
    X = np.asarray(event_emb, np.float32)
    n = X.shape[0]
    assert n <= NPAD

    ei = np.asarray(edge_index, np.int64)
    src = np.concatenate([ei[0], np.arange(n, dtype=np.int64)])
    dst = np.concatenate([ei[1], np.arange(n, dtype=np.int64)])

    deg = np.bincount(dst, minlength=NPAD).astype(np.float32)
    dinv = np.where(deg > 0, 1.0 / np.sqrt(deg), 0.0).astype(np.float32)

    # dense per-block-pair count masks: mask[c, b, s, half, g', j].
    # The src-block axis is permuted into AllGather-section order
    # (dst-sub-block 0-4 | 5-8 | 9) so phase 2A fills table slots in plain
    # slot order as each collective slice lands; slot 79 is the all-padding
    # block (global 79) and is skipped everywhere.
    mask = np.zeros((N_CORES, NBLK, 128, NGB, 128), np.float32)
    c = dst // PER
    b = (dst % PER) // 128
    j = dst % 128
    s = src % 128
    g = src // 128
    np.add.at(mask, (c, b, s, g, j), 1.0)
    mask = mask[:, :, :, GLIST, :].reshape(N_CORES, NBLK, 128, 2, HALF_G, 128)

    W1 = np.asarray(W1, np.float32)
    W2 = np.asarray(W2, np.float32)
    v1 = W2 @ np.asarray(att_src, np.float32)
    v2 = W2 @ np.asarray(att_dst, np.float32)

    Xp = np.zeros((NPAD, D), np.float32)
    Xp[:n] = X
    xt = _bf16(Xp.T.reshape(2, 128, NPAD))

    W2p = np.zeros((D, TW), np.float32)
    W2p[:, :D] = W2
    W2p[:, 256] = v1

    dinv_cols = dinv.reshape(NGB, 128).T          # [128, 80] global order
    dinva = np.ascontiguousarray(dinv_cols[:, GLIST])  # slot order (src side)

    shared = dict(
        xt=xt,
        w1=_bf16(W1.reshape(2, 128, D)),
        w2p=_bf16(W2p.reshape(2, 128, TW)),
        v2c=_bf16(v2.reshape(2, 128, 1)),
        b1r=np.ascontiguousarray(np.tile(np.asarray(b1, np.float32)[None], (128, 1))),
        b2r=np.ascontiguousarray(np.tile(np.asarray(b2, np.float32)[None], (128, 1))),
        dinva=dinva,
        ident=_bf16(np.eye(128, dtype=np.float32)),
        ones1=_bf16(np.ones((1, 128), np.float32)),
    )
    per_core = [
        dict(
            mask=_bf16(mask[cc]),
            dinvl=np.ascontiguousarray(dinv_cols[:, cc * NBLK:(cc + 1) * NBLK]),
        )
        for cc in range(N_CORES)
    ]
    return shared, per_core, n


# ----------------------------------------------------------------------------
# device program
# ----------------------------------------------------------------------------

def _build_nc(use_collective=True):
    nc = bacc.Bacc(
        "TRN2", target_bir_lowering=False, debug=False, num_devices=N_CORES
    )

    xt_d = nc.dram_tensor("xt", [2, 128, NPAD], BF16, kind="ExternalInput")
    w1_d = nc.dram_tensor("w1", [2, 128, D], BF16, kind="ExternalInput")
    w2p_d = nc.dram_tensor("w2p", [2, 128, TW], BF16, kind="ExternalInput")
    v2c_d = nc.dram_tensor("v2c", [2, 128, 1], BF16, kind="ExternalInput")
    b1r_d = nc.dram_tensor("b1r", [128, D], FP32, kind="ExternalInput")
    b2r_d = nc.dram_tensor("b2r", [128, D], FP32, kind="ExternalInput")
    dinva_d = nc.dram_tensor("dinva", [128, NGB], FP32, kind="ExternalInput")
    ident_d = nc.dram_tensor("ident", [128, 128], BF16, kind="ExternalInput")
    ones1_d = nc.dram_tensor("ones1", [1, 128], BF16, kind="ExternalInput")
    mask_d = nc.dram_tensor(
        "mask", [NBLK, 128, 2, HALF_G, 128], BF16, kind="ExternalInput"
    )
    dinvl_d = nc.dram_tensor("dinvl", [128, NBLK], FP32, kind="ExternalInput")
    out_d = nc.dram_tensor("out_slice", [PER, D], FP32, kind="ExternalOutput")

    # three staggered AllGather slices: blocks 0-4 | 5-8 | 9 -- the earlier
    # (larger) ones overlap layer-1 compute, the last is tiny
    AG_SPLITS = AG_BLOCKS
    ht_slice_s, ht_full_s = [], []
    for si, (b0, b1) in enumerate(AG_SPLITS):
        w = (b1 - b0) * 128
        ht_slice_s.append(
            nc.dram_tensor(f"ht_slice_{si}", [2, 128, w], BF16)
        )
        ht_full_s.append(
            nc.dram_tensor(
                f"ht_full_{si}", [N_CORES, 2, 128, w], BF16,
                addr_space="Shared",
            )
        )

    mu, ad, mx = mybir.AluOpType.mult, mybir.AluOpType.add, mybir.AluOpType.max

    with tile.TileContext(nc) as tc:
        with tc.tile_pool(name="const", bufs=1) as cp:
            # w1 on the sync ring (needed first, with xt); the rest on the
            # scalar ring so they don't delay the xt stream
            w1_sb = cp.tile([128, 2, D], BF16)
            for k in range(2):
                nc.sync.dma_start(w1_sb[:, k, :], w1_d[k])
            dinva_sb = cp.tile([128, NGB], FP32)
            nc.scalar.dma_start(dinva_sb[:], dinva_d[:, :])
            ident_sb = cp.tile([128, 128], BF16)
            nc.scalar.dma_start(ident_sb[:], ident_d[:, :])
            ones1_sb = cp.tile([1, 128], BF16)
            nc.scalar.dma_start(ones1_sb[:], ones1_d[:, :])
            b1_sb = cp.tile([128, D], FP32)
            nc.scalar.dma_start(b1_sb[:], b1r_d[:, :])
            b2_sb = cp.tile([128, D], FP32)
            nc.scalar.dma_start(b2_sb[:], b2r_d[:, :])
            dinvl_sb = cp.tile([128, NBLK], FP32)
            nc.scalar.dma_start(dinvl_sb[:], dinvl_d[:, :])
            w2_sb = cp.tile([128, 2, TW], BF16)
            v2_sb = cp.tile([128, 2, 1], BF16)
            for k in range(2):
                nc.scalar.dma_start(w2_sb[:, k, :], w2p_d[k])
                nc.scalar.dma_start(v2_sb[:, k, :], v2c_d[k])

            # tensors that live across phases
            with tc.tile_pool(name="persist", bufs=1) as pper:
                table2_sb = pper.tile([128, NGB, TW], BF16)
                l2bc_sb = pper.tile([128, NBLK, 128], FP16)
                l1grid_sb = pper.tile([128, NGB, 128], FP16)
                l1h_sb = pper.tile([128, NGB], FP16)
                # H1^T for the last AG slice (dst sub-blocks 7-9) -- small,
                # persistent so 2A's tail can interleave with layer-2 start
                ht3_sb = pper.tile([128, 2 * N_CORES, 3 * 128], BF16)

                with tc.tile_pool(name="xw1_p", bufs=1) as xwp:
                    xw1_sb = xwp.tile([128, NGB, D], BF16)

                    # ---- phase 1A: table1 = dinv[u] * (X @ W1), SBUF ----
                    half = NPAD // 2
                    with (
                        tc.tile_pool(name="xt_p", bufs=2) as xp,
                        tc.psum_pool(name="ps1_p", bufs=2) as pp1,
                    ):
                        for hh in range(2):
                            xt_sb = xp.tile([128, 2, half], BF16, tag="xt")
                            for k in range(2):
                                nc.sync.dma_start(
                                    xt_sb[:, k, :],
                                    xt_d[k, :, hh * half:(hh + 1) * half],
                                )
                            for jj in range(half // 128):
                                g = hh * (half // 128) + jj
                                if g == NGB - 1:
                                    continue  # all-padding src block
                                slot = POS[g]
                                ps = pp1.tile([128, D], FP32, tag="ps1")
                                for k in range(2):
                                    nc.tensor.matmul(
                                        ps[:],
                                        lhsT=xt_sb[:, k, jj * 128:(jj + 1) * 128],
                                        rhs=w1_sb[:, k, :],
                                        start=(k == 0),
                                        stop=(k == 1),
                                    )
                                # dinv-scaled psum->table copy, alternating
                                # engines so the copy stream keeps up with PE
                                if slot % 2 == 0:
                                    nc.scalar.mul(
                                        xw1_sb[:, slot, :], ps[:],
                                        dinva_sb[:, slot:slot + 1],
                                    )
                                else:
                                    nc.vector.tensor_scalar(
                                        xw1_sb[:, slot, :], ps[:],
                                        dinva_sb[:, slot:slot + 1], None, op0=mu,
                                    )

                    # table2 fixed column 256 -> 1.0 (denominator)
                    nc.vector.memset(table2_sb[:, :, 256:257], 1.0)

                    # ht_sb allocated before the L1 pools so its region is
                    # free early and the post-AllGather loads overlap L1
                    with tc.tile_pool(name="ht2_p", bufs=1) as hp2:
                        # H1^T for AG slices 0-1 (dst sub-blocks 0-6)
                        ht_sb = hp2.tile([128, 2 * N_CORES, 7 * 128], BF16)

                        # ---- phase 1B: GCN aggregate + H1^T + local l2 ----
                        with (
                            tc.tile_pool(name="m1_p", bufs=3) as mp,
                            tc.tile_pool(name="h1_p", bufs=2) as hp,
                            tc.tile_pool(name="ht_p", bufs=1) as htp,
                            tc.tile_pool(name="l2r_p", bufs=2) as lrp,
                            tc.psum_pool(name="psa_p", bufs=2) as ppa,
                            tc.psum_pool(name="pst_p", bufs=2) as ppt,
                        ):
                            ht_st = htp.tile([128, 2, PER], BF16)
                            for b in range(NBLK):
                                psa = ppa.tile([128, D], FP32, tag="agg1")
                                for hh in range(2):
                                    mt = mp.tile(
                                        [128, HALF_G, 128], BF16, tag="m1s"
                                    )
                                    nc.sync.dma_start(mt[:], mask_d[b, :, hh])
                                    ng = HALF_G - (1 if hh == 1 else 0)
                                    for gg in range(ng):
                                        g = hh * HALF_G + gg
                                        nc.tensor.matmul(
                                            psa[:],
                                            lhsT=mt[:, gg, :],
                                            rhs=xw1_sb[:, g, :],
                                            start=(g == 0),
                                            stop=(g == NGB - 2),
                                        )
                                h1 = hp.tile([128, D], BF16, tag="h1")
                                nc.vector.scalar_tensor_tensor(
                                    h1[:], psa[:], dinvl_sb[:, b:b + 1], b1_sb[:],
                                    op0=mu, op1=ad,
                                )
                                nc.vector.tensor_scalar_max(h1[:], h1[:], 0.0)
                                for k in range(2):
                                    ptt = ppt.tile([128, 128], BF16, tag="pt")
                                    nc.tensor.transpose(
                                        ptt[:], h1[:, k * 128:(k + 1) * 128],
                                        ident_sb[:],
                                    )
                                    nc.vector.tensor_copy(
                                        ht_st[:, k, b * 128:(b + 1) * 128], ptt[:]
                                    )
                                # local dst logits l2 for this block
                                l2ps = ppt.tile([128, 128], FP32, tag="l2ps")
                                for k in range(2):
                                    nc.tensor.matmul(
                                        l2ps[0:1, :],
                                        lhsT=v2_sb[:, k, :],
                                        rhs=ht_st[:, k, b * 128:(b + 1) * 128],
                                        start=(k == 0),
                                        stop=(k == 1),
                                    )
                                l2row = lrp.tile([1, 128], BF16, tag="l2row")
                                nc.vector.tensor_copy(l2row[:], l2ps[0:1, :])
                                bcps = ppt.tile([128, 128], FP32, tag="bcps")
                                nc.tensor.matmul(
                                    bcps[:], lhsT=ones1_sb[:], rhs=l2row[:],
                                    start=True, stop=True,
                                )
                                nc.vector.tensor_copy(l2bc_sb[:, b, :], bcps[:])
                                # staggered AllGathers fire as soon as their
                                # block range is transposed (scalar DMA ring
                                # jumps the mask-stream queue)
                                for si, (b0, b1) in enumerate(AG_SPLITS):
                                    if b != b1 - 1:
                                        continue
                                    lo, w = b0 * 128, (b1 - b0) * 128
                                    for k in range(2):
                                        nc.scalar.dma_start(
                                            ht_slice_s[si][k],
                                            ht_st[:, k, lo:lo + w],
                                        )
                                    if use_collective:
                                        nc.gpsimd.collective_compute(
                                            "AllGather",
                                            mybir.AluOpType.bypass,
                                            replica_groups=[list(range(N_CORES))],
                                            ins=[ht_slice_s[si][:, :, :]],
                                            outs=[ht_full_s[si][:, :, :, :]],
                                        )
                                    else:
                                        for r in range(N_CORES):
                                            nc.sync.dma_start(
                                                ht_full_s[si][r],
                                                ht_slice_s[si][:, :, :],
                                            )
                                    for r in range(N_CORES):
                                        for k in range(2):
                                            if si < 2:
                                                nc.scalar.dma_start(
                                                    ht_sb[:, 2 * r + k,
                                                          lo:lo + w],
                                                    ht_full_s[si][r, k],
                                                )
                                            else:
                                                nc.scalar.dma_start(
                                                    ht3_sb[:, 2 * r + k, :],
                                                    ht_full_s[si][r, k],
                                                )

                        # ---- phase 2A sections 1-2 (slots 0..55) ----
                        # slots are AllGather-section-ordered by GLIST, so a
                        # plain slot loop consumes each collective slice as
                        # it lands
                        def do_2a_slot(slot, pool, src_sb, base_bb):
                            g = GLIST[slot]
                            r, bb = divmod(g, NBLK)
                            ps = pool.tile(
                                [128, TW], FP32, tag="ps3", name=f"ps3_{slot}"
                            )
                            o = (bb - base_bb) * 128
                            for k in range(2):
                                nc.tensor.matmul(
                                    ps[:],
                                    lhsT=src_sb[:, 2 * r + k, o:o + 128],
                                    rhs=w2_sb[:, k, :],
                                    start=(k == 0),
                                    stop=(k == 1),
                                )
                            if slot % 2 == 0:
                                nc.scalar.copy(
                                    table2_sb[:, slot, 0:256], ps[:, 0:256]
                                )
                            else:
                                nc.vector.tensor_copy(
                                    table2_sb[:, slot, 0:256], ps[:, 0:256]
                                )
                            nc.vector.tensor_copy(
                                l1h_sb[:, slot:slot + 1], ps[:, 256:257]
                            )

                        with tc.psum_pool(name="ps3_p", bufs=2) as pp3:
                            for slot in range(SEC_SLOTS[0] + SEC_SLOTS[1]):
                                do_2a_slot(slot, pp3, ht_sb, 0)

                # l1 source-logit grid for the first layer-2 chunk
                nc.vector.tensor_copy(
                    l1grid_sb[:, 0:HALF_G, :],
                    l1h_sb[:, 0:HALF_G].unsqueeze(-1)
                    .broadcast_to([128, HALF_G, 128]),
                )

                # ---- phase 2B: GAT aggregate ----
                # Software-pipelined: engines execute their queues in FIFO
                # order, so producers for chunk i are issued before the
                # consumers of chunk i-1, and block outputs lag one more
                # step -- no engine ever heads its queue with a
                # not-yet-satisfiable dependency.
                with (
                    tc.tile_pool(name="m2m_p", bufs=3) as mp2,
                    tc.tile_pool(name="z_p", bufs=2) as zp,
                    tc.tile_pool(name="e_p", bufs=2) as ep,
                    tc.tile_pool(name="a2_p", bufs=2) as ap2,
                    tc.tile_pool(name="o_p", bufs=2) as op_,
                    tc.tile_pool(name="rc_p", bufs=2) as rcp,
                    tc.psum_pool(name="ps4_p", bufs=2) as pp4,
                ):
                    chunks = [(b, hh) for b in range(NBLK) for hh in range(2)]
                    ps_of = {}
                    pending = []  # produced, not yet consumed
                    done_ps = []  # psum tiles awaiting output emission

                    def produce(b, hh):
                        ng = HALF_G - (1 if hh == 1 else 0)
                        mt = mp2.tile([128, HALF_G, 128], BF16, tag="m2s")
                        nc.sync.dma_start(mt[:], mask_d[b, :, hh])
                        z = zp.tile([128, HALF_G, 128], FP16, tag="z")
                        nc.vector.tensor_tensor(
                            z[:, :ng, :],
                            l1grid_sb[:, hh * HALF_G:hh * HALF_G + ng, :],
                            l2bc_sb[:, b:b + 1, :].broadcast_to([128, ng, 128]),
                            op=ad,
                        )
                        # exp(leakyrelu(z)) == max(e^z, e^(0.2 z)) exactly
                        e1 = ep.tile([128, HALF_G, 128], BF16, tag="e1")
                        nc.scalar.activation(
                            e1[:, :ng, :], z[:, :ng, :],
                            mybir.ActivationFunctionType.Exp,
                        )
                        e2 = ep.tile([128, HALF_G, 128], BF16, tag="e2")
                        nc.scalar.activation(
                            e2[:, :ng, :], z[:, :ng, :],
                            mybir.ActivationFunctionType.Exp, scale=0.2,
                        )
                        return (b, hh, ng, mt, e1, e2)

                    def consume(b, hh, ng, mt, e1, e2):
                        nc.vector.tensor_tensor(
                            e1[:, :ng, :], e1[:, :ng, :], e2[:, :ng, :], op=mx
                        )
                        m2 = ap2.tile([128, HALF_G, 128], BF16, tag="m2")
                        nc.vector.tensor_tensor(
                            m2[:, :ng, :], e1[:, :ng, :], mt[:, :ng, :], op=mu
                        )
                        if hh == 0:
                            ps_of[b] = pp4.tile(
                                [128, TW], FP32, tag="agg2", name=f"agg2_{b}"
                            )
                        ps = ps_of[b]
                        for gg in range(ng):
                            g = hh * HALF_G + gg
                            nc.tensor.matmul(
                                ps[:],
                                lhsT=m2[:, gg, :],
                                rhs=table2_sb[:, g, :],
                                start=(g == 0),
                                stop=(g == NGB - 2),
                            )
                        if hh == 1:
                            done_ps.append((b, ps))

                    def emit_output():
                        b, ps = done_ps.pop(0)
                        rc = rcp.tile([128, 1], FP32, tag="rc")
                        nc.vector.reciprocal(rc[:], ps[:, 256:257])
                        ob = op_.tile([128, D], FP32, tag="ob")
                        nc.vector.scalar_tensor_tensor(
                            ob[:], ps[:, 0:D], rc[:], b2_sb[:], op0=mu, op1=ad,
                        )
                        nc.vector.tensor_scalar_max(ob[:], ob[:], 0.0)
                        nc.sync.dma_start(out_d[b * 128:(b + 1) * 128, :], ob[:])

                    # warm-up: first chunk's producers interleave with 2A's
                    # tail (section 3) and the second l1grid copy
                    pending.append(produce(0, 0))
                    with tc.psum_pool(name="ps3b_p", bufs=2) as pp3b:
                        for slot in range(SEC_SLOTS[0] + SEC_SLOTS[1], NGB - 1):
                            do_2a_slot(slot, pp3b, ht3_sb, NBLK - 3)
                    nc.vector.tensor_copy(
                        l1grid_sb[:, HALF_G:NGB - 1, :],
                        l1h_sb[:, HALF_G:NGB - 1].unsqueeze(-1)
                        .broadcast_to([128, HALF_G - 1, 128]),
                    )
                    for i, (b, hh) in enumerate(chunks[1:]):
                        pending.append(produce(b, hh))
                        consume(*pending.pop(0))
                        if i >= 2 and (i % 2) == 0 and done_ps:
                            emit_output()
                    consume(*pending.pop(0))
                    while done_ps:
                        emit_output()
    nc.finalize()
    return nc


# ----------------------------------------------------------------------------
# entry point
# ----------------------------------------------------------------------------

_CACHE = {}


def _get_nc():
    if "nc" not in _CACHE:
        _CACHE["nc"] = _build_nc()
    return _CACHE["nc"]


def kernel(event_emb, edge_index, W1, b1, W2, att_src, att_dst, b2,
           _want_results=False, _trace=False):
    shared, per_core, n = _prep(
        event_emb, edge_index, W1, b1, W2, att_src, att_dst, b2
    )
    nc = _get_nc()
    in_maps = [{**shared, **per_core[c]} for c in range(N_CORES)]
    res = run_bass_kernel_spmd(
        nc, in_maps, core_ids=list(range(N_CORES)), trace=_trace
    )
    out = np.concatenate(
        [res.results[c]["out_slice"] for c in range(N_CORES)], axis=0
    )[:n]
    if _want_results:
        return out, res
    return out


# revision 48
# speedup vs baseline: 1.1917x; 1.1917x over previous
"""Trainium2 Bass kernel for EventDiffusion GNN (GCNConv + GATConv, 2 layers).

Sharding: nodes partitioned into 8 contiguous ranges (one per NeuronCore);
each core aggregates messages for its 1280 destination nodes.  Layer-1
hidden states are exchanged with one AllGather (bf16) so every core can
build the full layer-2 feature table locally.

Aggregation strategy (both layers): dense block-push matmuls.  For every
(dst-block b, src-block g) pair a [128 src-slot x 128 dst-slot] count
matrix is streamed from HBM (bf16, contiguous) and used as the stationary
matmul operand against the SBUF-resident feature-table block:
    psum[b] += mask[b,g]^T @ table[g]            (80 matmuls per block)
No dma_gather anywhere (gather descriptor emission was the old bottleneck).

GCN normalization is folded node-wise: table1 rows are pre-scaled by
dinv[src] and the psum is post-scaled by dinv[dst], so layer 1 uses the
raw count mask directly.

GAT attention: alpha[s,d] = exp(leakyrelu(l1[s]+l2[d])) un-normalized --
the softmax shift is unnecessary because the final division by the
aggregated denominator makes the result scale-invariant per destination.
The attention matrix for a dst block is built block-wise on the Vector
engine from the rank-1 structure z[s,j] = l1[s] + l2b[j]:
    z (fp16) -> leakyrelu (1 fused op) -> exp (Scalar engine) -> * mask
then used as the push-matmul stationary operand.  The denominator is
accumulated through an all-ones column in the feature table.
"""

import numpy as np
import ml_dtypes

import concourse.bass as bass
import concourse.bacc as bacc
import concourse.mybir as mybir
import concourse.tile as tile
from concourse.bass_utils import run_bass_kernel_spmd

FP32 = mybir.dt.float32
BF16 = mybir.dt.bfloat16
FP16 = mybir.dt.float16

N_CORES = 8
D = 256
NPAD = 10240            # padded node count (80 blocks of 128)
PER = NPAD // N_CORES   # 1280 nodes per core
NBLK = PER // 128       # 10 dst blocks per core
NGB = NPAD // 128       # 80 src blocks (global)
HALF_G = NGB // 2       # src blocks per build chunk
CW = HALF_G * 128       # 5120 free-dim elems per chunk
TW = 257                # table2 width: 256 feats | 256: ones (denominator)
                        # (in the 2A psum, col 256 instead carries l1 = xw2@v1)

# src-block slot permutation: sections aligned with the three AllGather
# slices over dst sub-blocks (0-4, 5-8, 9); global block 79 lands in slot 79
AG_BLOCKS = [(0, 5), (5, 9), (9, 10)]
GLIST = [g for b0, b1 in AG_BLOCKS for g in range(NGB) if b0 <= g % NBLK < b1]
POS = {g: i for i, g in enumerate(GLIST)}

BF = ml_dtypes.bfloat16


def _bf16(a):
    return np.ascontiguousarray(np.asarray(a, np.float32).astype(BF))


# ----------------------------------------------------------------------------
# host-side preprocessing (graph metadata -> dense block count-masks)
# ----------------------------------------------------------------------------

def _prep(event_emb, edge_index, W1, b1, W2, att_src, att_dst, b2):
    X = np.asarray(event_emb, np.float32)
    n = X.shape[0]
    assert n <= NPAD

    ei = np.asarray(edge_index, np.int64)
    src = np.concatenate([ei[0], np.arange(n, dtype=np.int64)])
    dst = np.concatenate([ei[1], np.arange(n, dtype=np.int64)])

    deg = np.bincount(dst, minlength=NPAD).astype(np.float32)
    dinv = np.where(deg > 0, 1.0 / np.sqrt(deg), 0.0).astype(np.float32)

    # dense per-block-pair count masks: mask[c, b, s, half, g', j].
    # The src-block axis is permuted into AllGather-section order
    # (dst-sub-block 0-4 | 5-8 | 9) so phase 2A fills table slots in plain
    # slot order as each collective slice lands; slot 79 is the all-padding
    # block (global 79) and is skipped everywhere.
    mask = np.zeros((N_CORES, NBLK, 128, NGB, 128), np.float32)
    c = dst // PER
    b = (dst % PER) // 128
    j = dst % 128
    s = src % 128
    g = src // 128
    np.add.at(mask, (c, b, s, g, j), 1.0)
    mask = mask[:, :, :, GLIST, :].reshape(N_CORES, NBLK, 128, 2, HALF_G, 128)

    W1 = np.asarray(W1, np.float32)
    W2 = np.asarray(W2, np.float32)
    v1 = W2 @ np.asarray(att_src, np.float32)
    v2 = W2 @ np.asarray(att_dst, np.float32)

    Xp = np.zeros((NPAD, D), np.float32)
    Xp[:n] = X
    xt = _bf16(Xp.T.reshape(2, 128, NPAD))

    W2p = np.zeros((D, TW), np.float32)
    W2p[:, :D] = W2
    W2p[:, 256] = v1

    dinv_cols = dinv.reshape(NGB, 128).T          # [128, 80] global order
    dinva = np.ascontiguousarray(dinv_cols[:, GLIST])  # slot order (src side)

    shared = dict(
        xt=xt,
        w1=_bf16(W1.reshape(2, 128, D)),
        w2p=_bf16(W2p.reshape(2, 128, TW)),
        v2c=_bf16(v2.reshape(2, 128, 1)),
        b1r=np.ascontiguousarray(np.tile(np.asarray(b1, np.float32)[None], (128, 1))),
        b2r=np.ascontiguousarray(np.tile(np.asarray(b2, np.float32)[None], (128, 1))),
        dinva=dinva,
        ident=_bf16(np.eye(128, dtype=np.float32)),
        ones1=_bf16(np.ones((1, 128), np.float32)),
    )
    per_core = [
        dict(
            mask=_bf16(mask[cc]),
            dinvl=np.ascontiguousarray(dinv_cols[:, cc * NBLK:(cc + 1) * NBLK]),
        )
        for cc in range(N_CORES)
    ]
    return shared, per_core, n


# ----------------------------------------------------------------------------
# device program
# ----------------------------------------------------------------------------

def _build_nc(use_collective=True):
    nc = bacc.Bacc(
        "TRN2", target_bir_lowering=False, debug=False, num_devices=N_CORES
    )

    xt_d = nc.dram_tensor("xt", [2, 128, NPAD], BF16, kind="ExternalInput")
    w1_d = nc.dram_tensor("w1", [2, 128, D], BF16, kind="ExternalInput")
    w2p_d = nc.dram_tensor("w2p", [2, 128, TW], BF16, kind="ExternalInput")
    v2c_d = nc.dram_tensor("v2c", [2, 128, 1], BF16, kind="ExternalInput")
    b1r_d = nc.dram_tensor("b1r", [128, D], FP32, kind="ExternalInput")
    b2r_d = nc.dram_tensor("b2r", [128, D], FP32, kind="ExternalInput")
    dinva_d = nc.dram_tensor("dinva", [128, NGB], FP32, kind="ExternalInput")
    ident_d = nc.dram_tensor("ident", [128, 128], BF16, kind="ExternalInput")
    ones1_d = nc.dram_tensor("ones1", [1, 128], BF16, kind="ExternalInput")
    mask_d = nc.dram_tensor(
        "mask", [NBLK, 128, 2, HALF_G, 128], BF16, kind="ExternalInput"
    )
    dinvl_d = nc.dram_tensor("dinvl", [128, NBLK], FP32, kind="ExternalInput")
    out_d = nc.dram_tensor("out_slice", [PER, D], FP32, kind="ExternalOutput")

    # three staggered AllGather slices: blocks 0-4 | 5-8 | 9 -- the earlier
    # (larger) ones overlap layer-1 compute, the last is tiny
    AG_SPLITS = AG_BLOCKS
    ht_slice_s, ht_full_s = [], []
    for si, (b0, b1) in enumerate(AG_SPLITS):
        w = (b1 - b0) * 128
        ht_slice_s.append(
            nc.dram_tensor(f"ht_slice_{si}", [2, 128, w], BF16)
        )
        ht_full_s.append(
            nc.dram_tensor(
                f"ht_full_{si}", [N_CORES, 2, 128, w], BF16,
                addr_space="Shared",
            )
        )

    mu, ad, mx = mybir.AluOpType.mult, mybir.AluOpType.add, mybir.AluOpType.max

    with tile.TileContext(nc) as tc:
        with tc.tile_pool(name="const", bufs=1) as cp:
            # w1 on the sync ring (needed first, with xt); the rest on the
            # scalar ring so they don't delay the xt stream
            w1_sb = cp.tile([128, 2, D], BF16)
            for k in range(2):
                nc.sync.dma_start(w1_sb[:, k, :], w1_d[k])
            dinva_sb = cp.tile([128, NGB], FP32)
            nc.scalar.dma_start(dinva_sb[:], dinva_d[:, :])
            ident_sb = cp.tile([128, 128], BF16)
            nc.scalar.dma_start(ident_sb[:], ident_d[:, :])
            ones1_sb = cp.tile([1, 128], BF16)
            nc.scalar.dma_start(ones1_sb[:], ones1_d[:, :])
            b1_sb = cp.tile([128, D], FP32)
            nc.scalar.dma_start(b1_sb[:], b1r_d[:, :])
            b2_sb = cp.tile([128, D], FP32)
            nc.scalar.dma_start(b2_sb[:], b2r_d[:, :])
            dinvl_sb = cp.tile([128, NBLK], FP32)
            nc.scalar.dma_start(dinvl_sb[:], dinvl_d[:, :])
            w2_sb = cp.tile([128, 2, TW], BF16)
            v2_sb = cp.tile([128, 2, 1], BF16)
            for k in range(2):
                nc.scalar.dma_start(w2_sb[:, k, :], w2p_d[k])
                nc.scalar.dma_start(v2_sb[:, k, :], v2c_d[k])

            # tensors that live across phases
            with tc.tile_pool(name="persist", bufs=1) as pper:
                table2_sb = pper.tile([128, NGB, TW], BF16)
                l2bc_sb = pper.tile([128, NBLK, 128], FP16)
                l1grid_sb = pper.tile([128, NGB, 128], FP16)
                l1h_sb = pper.tile([128, NGB], FP16)

                with tc.tile_pool(name="xw1_p", bufs=1) as xwp:
                    xw1_sb = xwp.tile([128, NGB, D], BF16)

                    # ---- phase 1A: table1 = dinv[u] * (X @ W1), SBUF ----
                    half = NPAD // 2
                    with (
                        tc.tile_pool(name="xt_p", bufs=2) as xp,
                        tc.psum_pool(name="ps1_p", bufs=2) as pp1,
                    ):
                        for hh in range(2):
                            xt_sb = xp.tile([128, 2, half], BF16, tag="xt")
                            for k in range(2):
                                nc.sync.dma_start(
                                    xt_sb[:, k, :],
                                    xt_d[k, :, hh * half:(hh + 1) * half],
                                )
                            for jj in range(half // 128):
                                g = hh * (half // 128) + jj
                                if g == NGB - 1:
                                    continue  # all-padding src block
                                slot = POS[g]
                                ps = pp1.tile([128, D], FP32, tag="ps1")
                                for k in range(2):
                                    nc.tensor.matmul(
                                        ps[:],
                                        lhsT=xt_sb[:, k, jj * 128:(jj + 1) * 128],
                                        rhs=w1_sb[:, k, :],
                                        start=(k == 0),
                                        stop=(k == 1),
                                    )
                                # dinv-scaled psum->table copy, alternating
                                # engines so the copy stream keeps up with PE
                                if slot % 2 == 0:
                                    nc.scalar.mul(
                                        xw1_sb[:, slot, :], ps[:],
                                        dinva_sb[:, slot:slot + 1],
                                    )
                                else:
                                    nc.vector.tensor_scalar(
                                        xw1_sb[:, slot, :], ps[:],
                                        dinva_sb[:, slot:slot + 1], None, op0=mu,
                                    )

                    # table2 fixed column 256 -> 1.0 (denominator)
                    nc.vector.memset(table2_sb[:, :, 256:257], 1.0)

                    # ht_sb allocated before the L1 pools so its region is
                    # free early and the post-AllGather loads overlap L1
                    with tc.tile_pool(name="ht2_p", bufs=1) as hp2:
                        ht_sb = hp2.tile([128, 2 * N_CORES, PER], BF16)

                        # ---- phase 1B: GCN aggregate + H1^T + local l2 ----
                        with (
                            tc.tile_pool(name="m1_p", bufs=3) as mp,
                            tc.tile_pool(name="h1_p", bufs=2) as hp,
                            tc.tile_pool(name="ht_p", bufs=1) as htp,
                            tc.tile_pool(name="l2r_p", bufs=2) as lrp,
                            tc.psum_pool(name="psa_p", bufs=2) as ppa,
                            tc.psum_pool(name="pst_p", bufs=2) as ppt,
                        ):
                            ht_st = htp.tile([128, 2, PER], BF16)
                            for b in range(NBLK):
                                psa = ppa.tile([128, D], FP32, tag="agg1")
                                for hh in range(2):
                                    mt = mp.tile(
                                        [128, HALF_G, 128], BF16, tag="m1s"
                                    )
                                    nc.sync.dma_start(mt[:], mask_d[b, :, hh])
                                    ng = HALF_G - (1 if hh == 1 else 0)
                                    for gg in range(ng):
                                        g = hh * HALF_G + gg
                                        nc.tensor.matmul(
                                            psa[:],
                                            lhsT=mt[:, gg, :],
                                            rhs=xw1_sb[:, g, :],
                                            start=(g == 0),
                                            stop=(g == NGB - 2),
                                        )
                                h1 = hp.tile([128, D], BF16, tag="h1")
                                nc.vector.scalar_tensor_tensor(
                                    h1[:], psa[:], dinvl_sb[:, b:b + 1], b1_sb[:],
                                    op0=mu, op1=ad,
                                )
                                nc.vector.tensor_scalar_max(h1[:], h1[:], 0.0)
                                for k in range(2):
                                    ptt = ppt.tile([128, 128], BF16, tag="pt")
                                    nc.tensor.transpose(
                                        ptt[:], h1[:, k * 128:(k + 1) * 128],
                                        ident_sb[:],
                                    )
                                    nc.vector.tensor_copy(
                                        ht_st[:, k, b * 128:(b + 1) * 128], ptt[:]
                                    )
                                # local dst logits l2 for this block
                                l2ps = ppt.tile([128, 128], FP32, tag="l2ps")
                                for k in range(2):
                                    nc.tensor.matmul(
                                        l2ps[0:1, :],
                                        lhsT=v2_sb[:, k, :],
                                        rhs=ht_st[:, k, b * 128:(b + 1) * 128],
                                        start=(k == 0),
                                        stop=(k == 1),
                                    )
                                l2row = lrp.tile([1, 128], BF16, tag="l2row")
                                nc.vector.tensor_copy(l2row[:], l2ps[0:1, :])
                                bcps = ppt.tile([128, 128], FP32, tag="bcps")
                                nc.tensor.matmul(
                                    bcps[:], lhsT=ones1_sb[:], rhs=l2row[:],
                                    start=True, stop=True,
                                )
                                nc.vector.tensor_copy(l2bc_sb[:, b, :], bcps[:])
                                # staggered AllGathers fire as soon as their
                                # block range is transposed (scalar DMA ring
                                # jumps the mask-stream queue)
                                for si, (b0, b1) in enumerate(AG_SPLITS):
                                    if b != b1 - 1:
                                        continue
                                    lo, w = b0 * 128, (b1 - b0) * 128
                                    for k in range(2):
                                        nc.scalar.dma_start(
                                            ht_slice_s[si][k],
                                            ht_st[:, k, lo:lo + w],
                                        )
                                    if use_collective:
                                        nc.gpsimd.collective_compute(
                                            "AllGather",
                                            mybir.AluOpType.bypass,
                                            replica_groups=[list(range(N_CORES))],
                                            ins=[ht_slice_s[si][:, :, :]],
                                            outs=[ht_full_s[si][:, :, :, :]],
                                        )
                                    else:
                                        for r in range(N_CORES):
                                            nc.sync.dma_start(
                                                ht_full_s[si][r],
                                                ht_slice_s[si][:, :, :],
                                            )
                                    for r in range(N_CORES):
                                        for k in range(2):
                                            nc.scalar.dma_start(
                                                ht_sb[:, 2 * r + k, lo:lo + w],
                                                ht_full_s[si][r, k],
                                            )

                        # ---- phase 2A: table2 = [H1@W2 | 1], SBUF ----
                        # slots are AllGather-section-ordered by GLIST, so a
                        # plain slot loop consumes each collective slice as
                        # it lands
                        with tc.psum_pool(name="ps3_p", bufs=2) as pp3:
                            for slot in range(NGB - 1):
                                g = GLIST[slot]
                                r, bb = divmod(g, NBLK)
                                ps = pp3.tile([128, TW], FP32, tag="ps3")
                                for k in range(2):
                                    nc.tensor.matmul(
                                        ps[:],
                                        lhsT=ht_sb[
                                            :, 2 * r + k, bb * 128:(bb + 1) * 128
                                        ],
                                        rhs=w2_sb[:, k, :],
                                        start=(k == 0),
                                        stop=(k == 1),
                                    )
                                if slot % 2 == 0:
                                    nc.scalar.copy(
                                        table2_sb[:, slot, 0:256], ps[:, 0:256]
                                    )
                                else:
                                    nc.vector.tensor_copy(
                                        table2_sb[:, slot, 0:256], ps[:, 0:256]
                                    )
                                nc.vector.tensor_copy(
                                    l1h_sb[:, slot:slot + 1], ps[:, 256:257]
                                )

                # materialize l1 source-logit grid (fp16, broadcast over j)
                for hh in range(2):
                    ng = HALF_G - (1 if hh == 1 else 0)
                    nc.vector.tensor_copy(
                        l1grid_sb[:, hh * HALF_G:hh * HALF_G + ng, :],
                        l1h_sb[:, hh * HALF_G:hh * HALF_G + ng]
                        .unsqueeze(-1)
                        .broadcast_to([128, ng, 128]),
                    )

                # ---- phase 2B: GAT aggregate ----
                # Software-pipelined: engines execute their queues in FIFO
                # order, so producers for chunk i are issued before the
                # consumers of chunk i-1, and block outputs lag one more
                # step -- no engine ever heads its queue with a
                # not-yet-satisfiable dependency.
                with (
                    tc.tile_pool(name="m2m_p", bufs=3) as mp2,
                    tc.tile_pool(name="z_p", bufs=2) as zp,
                    tc.tile_pool(name="e_p", bufs=2) as ep,
                    tc.tile_pool(name="a2_p", bufs=2) as ap2,
                    tc.tile_pool(name="o_p", bufs=2) as op_,
                    tc.tile_pool(name="rc_p", bufs=2) as rcp,
                    tc.psum_pool(name="ps4_p", bufs=2) as pp4,
                ):
                    chunks = [(b, hh) for b in range(NBLK) for hh in range(2)]
                    ps_of = {}
                    pending = []  # produced, not yet consumed
                    done_ps = []  # psum tiles awaiting output emission

                    def produce(b, hh):
                        ng = HALF_G - (1 if hh == 1 else 0)
                        mt = mp2.tile([128, HALF_G, 128], BF16, tag="m2s")
                        nc.sync.dma_start(mt[:], mask_d[b, :, hh])
                        z = zp.tile([128, HALF_G, 128], FP16, tag="z")
                        nc.vector.tensor_tensor(
                            z[:, :ng, :],
                            l1grid_sb[:, hh * HALF_G:hh * HALF_G + ng, :],
                            l2bc_sb[:, b:b + 1, :].broadcast_to([128, ng, 128]),
                            op=ad,
                        )
                        # exp(leakyrelu(z)) == max(e^z, e^(0.2 z)) exactly
                        e1 = ep.tile([128, HALF_G, 128], BF16, tag="e1")
                        nc.scalar.activation(
                            e1[:, :ng, :], z[:, :ng, :],
                            mybir.ActivationFunctionType.Exp,
                        )
                        e2 = ep.tile([128, HALF_G, 128], BF16, tag="e2")
                        nc.scalar.activation(
                            e2[:, :ng, :], z[:, :ng, :],
                            mybir.ActivationFunctionType.Exp, scale=0.2,
                        )
                        return (b, hh, ng, mt, e1, e2)

                    def consume(b, hh, ng, mt, e1, e2):
                        nc.vector.tensor_tensor(
                            e1[:, :ng, :], e1[:, :ng, :], e2[:, :ng, :], op=mx
                        )
                        m2 = ap2.tile([128, HALF_G, 128], BF16, tag="m2")
                        nc.vector.tensor_tensor(
                            m2[:, :ng, :], e1[:, :ng, :], mt[:, :ng, :], op=mu
                        )
                        if hh == 0:
                            ps_of[b] = pp4.tile(
                                [128, TW], FP32, tag="agg2", name=f"agg2_{b}"
                            )
                        ps = ps_of[b]
                        for gg in range(ng):
                            g = hh * HALF_G + gg
                            nc.tensor.matmul(
                                ps[:],
                                lhsT=m2[:, gg, :],
                                rhs=table2_sb[:, g, :],
                                start=(g == 0),
                                stop=(g == NGB - 2),
                            )
                        if hh == 1:
                            done_ps.append((b, ps))

                    def emit_output():
                        b, ps = done_ps.pop(0)
                        rc = rcp.tile([128, 1], FP32, tag="rc")
                        nc.vector.reciprocal(rc[:], ps[:, 256:257])
                        ob = op_.tile([128, D], FP32, tag="ob")
                        nc.vector.scalar_tensor_tensor(
                            ob[:], ps[:, 0:D], rc[:], b2_sb[:], op0=mu, op1=ad,
                        )
                        nc.vector.tensor_scalar_max(ob[:], ob[:], 0.0)
                        nc.sync.dma_start(out_d[b * 128:(b + 1) * 128, :], ob[:])

                    for i, (b, hh) in enumerate(chunks):
                        pending.append(produce(b, hh))
                        if len(pending) > 1:
                            consume(*pending.pop(0))
                        if i >= 3 and (i % 2) == 1 and done_ps:
                            emit_output()
                    consume(*pending.pop(0))
                    while done_ps:
                        emit_output()
    nc.finalize()
    return nc


# ----------------------------------------------------------------------------
# entry point
# ----------------------------------------------------------------------------

_CACHE = {}


def _get_nc():
    if "nc" not in _CACHE:
        _CACHE["nc"] = _build_nc()
    return _CACHE["nc"]


def kernel(event_emb, edge_index, W1, b1, W2, att_src, att_dst, b2,
           _want_results=False, _trace=False):
    shared, per_core, n = _prep(
        event_emb, edge_index, W1, b1, W2, att_src, att_dst, b2
    )
    nc = _get_nc()
    in_maps = [{**shared, **per_core[c]} for c in range(N_CORES)]
    res = run_bass_kernel_spmd(
        nc, in_maps, core_ids=list(range(N_CORES)), trace=_trace
    )
    out = np.concatenate(
        [res.results[c]["out_slice"] for c in range(N_CORES)], axis=0
    )[:n]
    if _want_results:
        return out, res
    return out


# revision 53
# speedup vs baseline: 1.2013x; 1.0080x over previous
"""Trainium2 Bass kernel for EventDiffusion GNN (GCNConv + GATConv, 2 layers).

Sharding: nodes partitioned into 8 contiguous ranges (one per NeuronCore);
each core aggregates messages for its 1280 destination nodes.  Layer-1
hidden states are exchanged with one AllGather (bf16) so every core can
build the full layer-2 feature table locally.

Aggregation strategy (both layers): dense block-push matmuls.  For every
(dst-block b, src-block g) pair a [128 src-slot x 128 dst-slot] count
matrix is streamed from HBM (bf16, contiguous) and used as the stationary
matmul operand against the SBUF-resident feature-table block:
    psum[b] += mask[b,g]^T @ table[g]            (80 matmuls per block)
No dma_gather anywhere (gather descriptor emission was the old bottleneck).

GCN normalization is folded node-wise: table1 rows are pre-scaled by
dinv[src] and the psum is post-scaled by dinv[dst], so layer 1 uses the
raw count mask directly.

GAT attention: alpha[s,d] = exp(leakyrelu(l1[s]+l2[d])) un-normalized --
the softmax shift is unnecessary because the final division by the
aggregated denominator makes the result scale-invariant per destination.
The attention matrix for a dst block is built block-wise on the Vector
engine from the rank-1 structure z[s,j] = l1[s] + l2b[j]:
    z (fp16) -> leakyrelu (1 fused op) -> exp (Scalar engine) -> * mask
then used as the push-matmul stationary operand.  The denominator is
accumulated through an all-ones column in the feature table.
"""

import numpy as np
import ml_dtypes

import concourse.bass as bass
import concourse.bacc as bacc
import concourse.mybir as mybir
import concourse.tile as tile
from concourse.bass_utils import run_bass_kernel_spmd

FP32 = mybir.dt.float32
BF16 = mybir.dt.bfloat16
FP16 = mybir.dt.float16
FP8 = mybir.dt.float8e4

N_CORES = 8
D = 256
NPAD = 10240            # padded node count (80 blocks of 128)
PER = NPAD // N_CORES   # 1280 nodes per core
NBLK = PER // 128       # 10 dst blocks per core
NGB = NPAD // 128       # 80 src blocks (global)
HALF_G = NGB // 2       # src blocks per build chunk
CW = HALF_G * 128       # 5120 free-dim elems per chunk
TW = 257                # table2 width: 256 feats | 256: ones (denominator)
                        # (in the 2A psum, col 256 instead carries l1 = xw2@v1)

# src-block slot permutation: sections aligned with the three AllGather
# slices over dst sub-blocks (0-4, 5-8, 9); global block 79 lands in slot 79
AG_BLOCKS = [(0, 5), (5, 9), (9, 10)]
GLIST = [g for b0, b1 in AG_BLOCKS for g in range(NGB) if b0 <= g % NBLK < b1]
POS = {g: i for i, g in enumerate(GLIST)}

BF = ml_dtypes.bfloat16


def _bf16(a):
    return np.ascontiguousarray(np.asarray(a, np.float32).astype(BF))


# ----------------------------------------------------------------------------
# host-side preprocessing (graph metadata -> dense block count-masks)
# ----------------------------------------------------------------------------

def _prep(event_emb, edge_index, W1, b1, W2, att_src, att_dst, b2):
    X = np.asarray(event_emb, np.float32)
    n = X.shape[0]
    assert n <= NPAD

    ei = np.asarray(edge_index, np.int64)
    src = np.concatenate([ei[0], np.arange(n, dtype=np.int64)])
    dst = np.concatenate([ei[1], np.arange(n, dtype=np.int64)])

    deg = np.bincount(dst, minlength=NPAD).astype(np.float32)
    dinv = np.where(deg > 0, 1.0 / np.sqrt(deg), 0.0).astype(np.float32)

    # dense per-block-pair count masks: mask[c, b, s, half, g', j].
    # The src-block axis is permuted into AllGather-section order
    # (dst-sub-block 0-4 | 5-8 | 9) so phase 2A fills table slots in plain
    # slot order as each collective slice lands; slot 79 is the all-padding
    # block (global 79) and is skipped everywhere.
    mask = np.zeros((N_CORES, NBLK, 128, NGB, 128), np.float32)
    c = dst // PER
    b = (dst % PER) // 128
    j = dst % 128
    s = src % 128
    g = src // 128
    np.add.at(mask, (c, b, s, g, j), 1.0)
    mask = mask[:, :, :, GLIST, :].reshape(N_CORES, NBLK, 128, 2, HALF_G, 128)

    W1 = np.asarray(W1, np.float32)
    W2 = np.asarray(W2, np.float32)
    v1 = W2 @ np.asarray(att_src, np.float32)
    v2 = W2 @ np.asarray(att_dst, np.float32)

    Xp = np.zeros((NPAD, D), np.float32)
    Xp[:n] = X
    xt = _bf16(Xp.T.reshape(2, 128, NPAD))

    W2p = np.zeros((D, TW), np.float32)
    W2p[:, :D] = W2
    W2p[:, 256] = v1

    dinv_cols = dinv.reshape(NGB, 128).T          # [128, 80] global order
    dinva = np.ascontiguousarray(dinv_cols[:, GLIST])  # slot order (src side)

    shared = dict(
        xt=xt,
        w1=_bf16(W1.reshape(2, 128, D)),
        w2p=_bf16(W2p.reshape(2, 128, TW)),
        v2c=_bf16(v2.reshape(2, 128, 1)),
        b1r=np.ascontiguousarray(np.tile(np.asarray(b1, np.float32)[None], (128, 1))),
        b2r=np.ascontiguousarray(np.tile(np.asarray(b2, np.float32)[None], (128, 1))),
        dinva=dinva,
        ident=_bf16(np.eye(128, dtype=np.float32)),
        ones1=_bf16(np.ones((1, 128), np.float32)),
    )
    per_core = [
        dict(
            mask=_bf16(mask[cc]),
            maskf8=np.ascontiguousarray(
                mask[cc].astype(ml_dtypes.float8_e4m3)
            ),
            dinvl=np.ascontiguousarray(dinv_cols[:, cc * NBLK:(cc + 1) * NBLK]),
        )
        for cc in range(N_CORES)
    ]
    return shared, per_core, n


# ----------------------------------------------------------------------------
# device program
# ----------------------------------------------------------------------------

def _build_nc(use_collective=True):
    nc = bacc.Bacc(
        "TRN2", target_bir_lowering=False, debug=False, num_devices=N_CORES
    )

    xt_d = nc.dram_tensor("xt", [2, 128, NPAD], BF16, kind="ExternalInput")
    w1_d = nc.dram_tensor("w1", [2, 128, D], BF16, kind="ExternalInput")
    w2p_d = nc.dram_tensor("w2p", [2, 128, TW], BF16, kind="ExternalInput")
    v2c_d = nc.dram_tensor("v2c", [2, 128, 1], BF16, kind="ExternalInput")
    b1r_d = nc.dram_tensor("b1r", [128, D], FP32, kind="ExternalInput")
    b2r_d = nc.dram_tensor("b2r", [128, D], FP32, kind="ExternalInput")
    dinva_d = nc.dram_tensor("dinva", [128, NGB], FP32, kind="ExternalInput")
    ident_d = nc.dram_tensor("ident", [128, 128], BF16, kind="ExternalInput")
    ones1_d = nc.dram_tensor("ones1", [1, 128], BF16, kind="ExternalInput")
    mask_d = nc.dram_tensor(
        "mask", [NBLK, 128, 2, HALF_G, 128], BF16, kind="ExternalInput"
    )
    maskf8_d = nc.dram_tensor(
        "maskf8", [NBLK, 128, 2, HALF_G, 128], FP8, kind="ExternalInput"
    )
    dinvl_d = nc.dram_tensor("dinvl", [128, NBLK], FP32, kind="ExternalInput")
    out_d = nc.dram_tensor("out_slice", [PER, D], FP32, kind="ExternalOutput")

    # three staggered AllGather slices: blocks 0-4 | 5-8 | 9 -- the earlier
    # (larger) ones overlap layer-1 compute, the last is tiny
    AG_SPLITS = AG_BLOCKS
    ht_slice_s, ht_full_s = [], []
    for si, (b0, b1) in enumerate(AG_SPLITS):
        w = (b1 - b0) * 128
        ht_slice_s.append(
            nc.dram_tensor(f"ht_slice_{si}", [2, 128, w], BF16)
        )
        ht_full_s.append(
            nc.dram_tensor(
                f"ht_full_{si}", [N_CORES, 2, 128, w], BF16,
                addr_space="Shared",
            )
        )

    mu, ad, mx = mybir.AluOpType.mult, mybir.AluOpType.add, mybir.AluOpType.max

    with tile.TileContext(nc) as tc:
        with tc.tile_pool(name="const", bufs=1) as cp:
            # w1 on the sync ring (needed first, with xt); the rest on the
            # scalar ring so they don't delay the xt stream
            w1_sb = cp.tile([128, 2, D], BF16)
            for k in range(2):
                nc.sync.dma_start(w1_sb[:, k, :], w1_d[k])
            dinva_sb = cp.tile([128, NGB], FP32)
            nc.scalar.dma_start(dinva_sb[:], dinva_d[:, :])
            ident_sb = cp.tile([128, 128], BF16)
            nc.scalar.dma_start(ident_sb[:], ident_d[:, :])
            ones1_sb = cp.tile([1, 128], BF16)
            nc.scalar.dma_start(ones1_sb[:], ones1_d[:, :])
            b1_sb = cp.tile([128, D], FP32)
            nc.scalar.dma_start(b1_sb[:], b1r_d[:, :])
            b2_sb = cp.tile([128, D], FP32)
            nc.scalar.dma_start(b2_sb[:], b2r_d[:, :])
            dinvl_sb = cp.tile([128, NBLK], FP32)
            nc.scalar.dma_start(dinvl_sb[:], dinvl_d[:, :])
            w2_sb = cp.tile([128, 2, TW], BF16)
            v2_sb = cp.tile([128, 2, 1], BF16)
            for k in range(2):
                nc.scalar.dma_start(w2_sb[:, k, :], w2p_d[k])
                nc.scalar.dma_start(v2_sb[:, k, :], v2c_d[k])

            # tensors that live across phases
            with tc.tile_pool(name="persist", bufs=1) as pper:
                table2_sb = pper.tile([128, NGB, TW], BF16)
                l2bc_sb = pper.tile([128, NBLK, 128], FP16)
                l1grid_sb = pper.tile([128, NGB, 128], FP16)
                l1h_sb = pper.tile([128, NGB], FP16)

                with tc.tile_pool(name="xw1_p", bufs=1) as xwp:
                    xw1_sb = xwp.tile([128, NGB, D], BF16)

                    # ---- phase 1A: table1 = dinv[u] * (X @ W1), SBUF ----
                    half = NPAD // 2
                    with (
                        tc.tile_pool(name="xt_p", bufs=2) as xp,
                        tc.psum_pool(name="ps1_p", bufs=2) as pp1,
                    ):
                        for hh in range(2):
                            xt_sb = xp.tile([128, 2, half], BF16, tag="xt")
                            for k in range(2):
                                nc.sync.dma_start(
                                    xt_sb[:, k, :],
                                    xt_d[k, :, hh * half:(hh + 1) * half],
                                )
                            for jj in range(half // 128):
                                g = hh * (half // 128) + jj
                                if g == NGB - 1:
                                    continue  # all-padding src block
                                slot = POS[g]
                                ps = pp1.tile([128, D], FP32, tag="ps1")
                                for k in range(2):
                                    nc.tensor.matmul(
                                        ps[:],
                                        lhsT=xt_sb[:, k, jj * 128:(jj + 1) * 128],
                                        rhs=w1_sb[:, k, :],
                                        start=(k == 0),
                                        stop=(k == 1),
                                    )
                                # dinv-scaled psum->table copy, alternating
                                # engines so the copy stream keeps up with PE
                                if slot % 2 == 0:
                                    nc.scalar.mul(
                                        xw1_sb[:, slot, :], ps[:],
                                        dinva_sb[:, slot:slot + 1],
                                    )
                                else:
                                    nc.vector.tensor_scalar(
                                        xw1_sb[:, slot, :], ps[:],
                                        dinva_sb[:, slot:slot + 1], None, op0=mu,
                                    )

                    # table2 fixed column 256 -> 1.0 (denominator)
                    nc.vector.memset(table2_sb[:, :, 256:257], 1.0)

                    # ht_sb allocated before the L1 pools so its region is
                    # free early and the post-AllGather loads overlap L1
                    with tc.tile_pool(name="ht2_p", bufs=1) as hp2:
                        ht_sb = hp2.tile([128, 2 * N_CORES, PER], BF16)

                        # ---- phase 1B: GCN aggregate + H1^T + local l2 ----
                        with (
                            tc.tile_pool(name="m1_p", bufs=3) as mp,
                            tc.tile_pool(name="h1_p", bufs=2) as hp,
                            tc.tile_pool(name="ht_p", bufs=1) as htp,
                            tc.tile_pool(name="l2r_p", bufs=2) as lrp,
                            tc.psum_pool(name="psa_p", bufs=2) as ppa,
                            tc.psum_pool(name="pst_p", bufs=2) as ppt,
                        ):
                            ht_st = htp.tile([128, 2, PER], BF16)
                            for b in range(NBLK):
                                psa = ppa.tile([128, D], FP32, tag="agg1")
                                for hh in range(2):
                                    mt = mp.tile(
                                        [128, HALF_G, 128], FP8, tag="m1s"
                                    )
                                    nc.sync.dma_start(mt[:], maskf8_d[b, :, hh])
                                    ng = HALF_G - (1 if hh == 1 else 0)
                                    for gg in range(ng):
                                        g = hh * HALF_G + gg
                                        nc.tensor.matmul(
                                            psa[:],
                                            lhsT=mt[:, gg, :],
                                            rhs=xw1_sb[:, g, :],
                                            start=(g == 0),
                                            stop=(g == NGB - 2),
                                        )
                                h1 = hp.tile([128, D], BF16, tag="h1")
                                nc.vector.scalar_tensor_tensor(
                                    h1[:], psa[:], dinvl_sb[:, b:b + 1], b1_sb[:],
                                    op0=mu, op1=ad,
                                )
                                nc.vector.tensor_scalar_max(h1[:], h1[:], 0.0)
                                for k in range(2):
                                    ptt = ppt.tile([128, 128], BF16, tag="pt")
                                    nc.tensor.transpose(
                                        ptt[:], h1[:, k * 128:(k + 1) * 128],
                                        ident_sb[:],
                                    )
                                    nc.vector.tensor_copy(
                                        ht_st[:, k, b * 128:(b + 1) * 128], ptt[:]
                                    )
                                # local dst logits l2 for this block
                                l2ps = ppt.tile([128, 128], FP32, tag="l2ps")
                                for k in range(2):
                                    nc.tensor.matmul(
                                        l2ps[0:1, :],
                                        lhsT=v2_sb[:, k, :],
                                        rhs=ht_st[:, k, b * 128:(b + 1) * 128],
                                        start=(k == 0),
                                        stop=(k == 1),
                                    )
                                l2row = lrp.tile([1, 128], BF16, tag="l2row")
                                nc.vector.tensor_copy(l2row[:], l2ps[0:1, :])
                                bcps = ppt.tile([128, 128], FP32, tag="bcps")
                                nc.tensor.matmul(
                                    bcps[:], lhsT=ones1_sb[:], rhs=l2row[:],
                                    start=True, stop=True,
                                )
                                nc.vector.tensor_copy(l2bc_sb[:, b, :], bcps[:])
                                # staggered AllGathers fire as soon as their
                                # block range is transposed (scalar DMA ring
                                # jumps the mask-stream queue)
                                for si, (b0, b1) in enumerate(AG_SPLITS):
                                    if b != b1 - 1:
                                        continue
                                    lo, w = b0 * 128, (b1 - b0) * 128
                                    for k in range(2):
                                        nc.scalar.dma_start(
                                            ht_slice_s[si][k],
                                            ht_st[:, k, lo:lo + w],
                                        )
                                    if use_collective:
                                        nc.gpsimd.collective_compute(
                                            "AllGather",
                                            mybir.AluOpType.bypass,
                                            replica_groups=[list(range(N_CORES))],
                                            ins=[ht_slice_s[si][:, :, :]],
                                            outs=[ht_full_s[si][:, :, :, :]],
                                        )
                                    else:
                                        for r in range(N_CORES):
                                            nc.sync.dma_start(
                                                ht_full_s[si][r],
                                                ht_slice_s[si][:, :, :],
                                            )
                                    for r in range(N_CORES):
                                        for k in range(2):
                                            nc.scalar.dma_start(
                                                ht_sb[:, 2 * r + k, lo:lo + w],
                                                ht_full_s[si][r, k],
                                            )

                        # ---- phase 2A: table2 = [H1@W2 | 1], SBUF ----
                        # slots are AllGather-section-ordered by GLIST, so a
                        # plain slot loop consumes each collective slice as
                        # it lands
                        with tc.psum_pool(name="ps3_p", bufs=2) as pp3:
                            for slot in range(NGB - 1):
                                g = GLIST[slot]
                                r, bb = divmod(g, NBLK)
                                ps = pp3.tile([128, TW], FP32, tag="ps3")
                                for k in range(2):
                                    nc.tensor.matmul(
                                        ps[:],
                                        lhsT=ht_sb[
                                            :, 2 * r + k, bb * 128:(bb + 1) * 128
                                        ],
                                        rhs=w2_sb[:, k, :],
                                        start=(k == 0),
                                        stop=(k == 1),
                                    )
                                if slot % 2 == 0:
                                    nc.scalar.copy(
                                        table2_sb[:, slot, 0:256], ps[:, 0:256]
                                    )
                                else:
                                    nc.vector.tensor_copy(
                                        table2_sb[:, slot, 0:256], ps[:, 0:256]
                                    )
                                nc.vector.tensor_copy(
                                    l1h_sb[:, slot:slot + 1], ps[:, 256:257]
                                )

                # materialize l1 source-logit grid (fp16, broadcast over j)
                for hh in range(2):
                    ng = HALF_G - (1 if hh == 1 else 0)
                    nc.vector.tensor_copy(
                        l1grid_sb[:, hh * HALF_G:hh * HALF_G + ng, :],
                        l1h_sb[:, hh * HALF_G:hh * HALF_G + ng]
                        .unsqueeze(-1)
                        .broadcast_to([128, ng, 128]),
                    )

                # ---- phase 2B: GAT aggregate ----
                # Software-pipelined: engines execute their queues in FIFO
                # order, so producers for chunk i are issued before the
                # consumers of chunk i-1, and block outputs lag one more
                # step -- no engine ever heads its queue with a
                # not-yet-satisfiable dependency.
                with (
                    tc.tile_pool(name="m2m_p", bufs=3) as mp2,
                    tc.tile_pool(name="z_p", bufs=2) as zp,
                    tc.tile_pool(name="e_p", bufs=3) as ep,
                    tc.tile_pool(name="a2_p", bufs=2) as ap2,
                    tc.tile_pool(name="o_p", bufs=2) as op_,
                    tc.tile_pool(name="rc_p", bufs=2) as rcp,
                    tc.psum_pool(name="ps4_p", bufs=3) as pp4,
                ):
                    chunks = [(b, hh) for b in range(NBLK) for hh in range(2)]
                    ps_of = {}
                    pending = []  # produced, not yet consumed
                    done_ps = []  # psum tiles awaiting output emission

                    def produce(b, hh):
                        ng = HALF_G - (1 if hh == 1 else 0)
                        mt = mp2.tile([128, HALF_G, 128], BF16, tag="m2s")
                        nc.sync.dma_start(mt[:], mask_d[b, :, hh])
                        z = zp.tile([128, HALF_G, 128], FP16, tag="z")
                        nc.vector.tensor_tensor(
                            z[:, :ng, :],
                            l1grid_sb[:, hh * HALF_G:hh * HALF_G + ng, :],
                            l2bc_sb[:, b:b + 1, :].broadcast_to([128, ng, 128]),
                            op=ad,
                        )
                        # exp(leakyrelu(z)) == max(e^z, e^(0.2 z)) exactly
                        e1 = ep.tile([128, HALF_G, 128], BF16, tag="e1")
                        nc.scalar.activation(
                            e1[:, :ng, :], z[:, :ng, :],
                            mybir.ActivationFunctionType.Exp,
                        )
                        e2 = ep.tile([128, HALF_G, 128], BF16, tag="e2")
                        nc.scalar.activation(
                            e2[:, :ng, :], z[:, :ng, :],
                            mybir.ActivationFunctionType.Exp, scale=0.2,
                        )
                        return (b, hh, ng, mt, e1, e2)

                    def consume(b, hh, ng, mt, e1, e2):
                        nc.vector.tensor_tensor(
                            e1[:, :ng, :], e1[:, :ng, :], e2[:, :ng, :], op=mx
                        )
                        m2 = ap2.tile([128, HALF_G, 128], BF16, tag="m2")
                        nc.vector.tensor_tensor(
                            m2[:, :ng, :], e1[:, :ng, :], mt[:, :ng, :], op=mu
                        )
                        if hh == 0:
                            ps_of[b] = pp4.tile(
                                [128, TW], FP32, tag="agg2", name=f"agg2_{b}"
                            )
                        ps = ps_of[b]
                        for gg in range(ng):
                            g = hh * HALF_G + gg
                            nc.tensor.matmul(
                                ps[:],
                                lhsT=m2[:, gg, :],
                                rhs=table2_sb[:, g, :],
                                start=(g == 0),
                                stop=(g == NGB - 2),
                            )
                        if hh == 1:
                            done_ps.append((b, ps))

                    def emit_output():
                        b, ps = done_ps.pop(0)
                        rc = rcp.tile([128, 1], FP32, tag="rc")
                        nc.vector.reciprocal(rc[:], ps[:, 256:257])
                        ob = op_.tile([128, D], FP32, tag="ob")
                        nc.vector.scalar_tensor_tensor(
                            ob[:], ps[:, 0:D], rc[:], b2_sb[:], op0=mu, op1=ad,
                        )
                        nc.vector.tensor_scalar_max(ob[:], ob[:], 0.0)
                        nc.sync.dma_start(out_d[b * 128:(b + 1) * 128, :], ob[:])

                    for i, (b, hh) in enumerate(chunks):
                        pending.append(produce(b, hh))
                        if len(pending) > 1:
                            consume(*pending.pop(0))
                        if i >= 3 and (i % 2) == 1 and done_ps:
                            emit_output()
                    consume(*pending.pop(0))
                    while done_ps:
                        emit_output()
    nc.finalize()
    return nc


# ----------------------------------------------------------------------------
# entry point
# ----------------------------------------------------------------------------

_CACHE = {}


def _get_nc():
    if "nc" not in _CACHE:
        _CACHE["nc"] = _build_nc()
    return _CACHE["nc"]


def kernel(event_emb, edge_index, W1, b1, W2, att_src, att_dst, b2,
           _want_results=False, _trace=False):
    shared, per_core, n = _prep(
        event_emb, edge_index, W1, b1, W2, att_src, att_dst, b2
    )
    nc = _get_nc()
    in_maps = [{**shared, **per_core[c]} for c in range(N_CORES)]
    res = run_bass_kernel_spmd(
        nc, in_maps, core_ids=list(range(N_CORES)), trace=_trace
    )
    out = np.concatenate(
        [res.results[c]["out_slice"] for c in range(N_CORES)], axis=0
    )[:n]
    if _want_results:
        return out, res
    return out


# revision 54
# speedup vs baseline: 1.2403x; 1.0324x over previous
"""Trainium2 Bass kernel for EventDiffusion GNN (GCNConv + GATConv, 2 layers).

Sharding: nodes partitioned into 8 contiguous ranges (one per NeuronCore);
each core aggregates messages for its 1280 destination nodes.  Layer-1
hidden states are exchanged with one AllGather (bf16) so every core can
build the full layer-2 feature table locally.

Aggregation strategy (both layers): dense block-push matmuls.  For every
(dst-block b, src-block g) pair a [128 src-slot x 128 dst-slot] count
matrix is streamed from HBM (bf16, contiguous) and used as the stationary
matmul operand against the SBUF-resident feature-table block:
    psum[b] += mask[b,g]^T @ table[g]            (80 matmuls per block)
No dma_gather anywhere (gather descriptor emission was the old bottleneck).

GCN normalization is folded node-wise: table1 rows are pre-scaled by
dinv[src] and the psum is post-scaled by dinv[dst], so layer 1 uses the
raw count mask directly.

GAT attention: alpha[s,d] = exp(leakyrelu(l1[s]+l2[d])) un-normalized --
the softmax shift is unnecessary because the final division by the
aggregated denominator makes the result scale-invariant per destination.
The attention matrix for a dst block is built block-wise on the Vector
engine from the rank-1 structure z[s,j] = l1[s] + l2b[j]:
    z (fp16) -> leakyrelu (1 fused op) -> exp (Scalar engine) -> * mask
then used as the push-matmul stationary operand.  The denominator is
accumulated through an all-ones column in the feature table.
"""

import numpy as np
import ml_dtypes

import concourse.bass as bass
import concourse.bacc as bacc
import concourse.mybir as mybir
import concourse.tile as tile
from concourse.bass_utils import run_bass_kernel_spmd

FP32 = mybir.dt.float32
BF16 = mybir.dt.bfloat16
FP16 = mybir.dt.float16
FP8 = mybir.dt.float8e4

N_CORES = 8
D = 256
NPAD = 10240            # padded node count (80 blocks of 128)
PER = NPAD // N_CORES   # 1280 nodes per core
NBLK = PER // 128       # 10 dst blocks per core
NGB = NPAD // 128       # 80 src blocks (global)
HALF_G = NGB // 2       # src blocks per build chunk
CW = HALF_G * 128       # 5120 free-dim elems per chunk
TW = 257                # table2 width: 256 feats | 256: ones (denominator)
                        # (in the 2A psum, col 256 instead carries l1 = xw2@v1)

# src-block slot permutation: sections aligned with the three AllGather
# slices over dst sub-blocks (0-4, 5-8, 9); global block 79 lands in slot 79
AG_BLOCKS = [(0, 5), (5, 9), (9, 10)]
GLIST = [g for b0, b1 in AG_BLOCKS for g in range(NGB) if b0 <= g % NBLK < b1]
POS = {g: i for i, g in enumerate(GLIST)}

BF = ml_dtypes.bfloat16


def _bf16(a):
    return np.ascontiguousarray(np.asarray(a, np.float32).astype(BF))


# ----------------------------------------------------------------------------
# host-side preprocessing (graph metadata -> dense block count-masks)
# ----------------------------------------------------------------------------

def _prep(event_emb, edge_index, W1, b1, W2, att_src, att_dst, b2):
    X = np.asarray(event_emb, np.float32)
    n = X.shape[0]
    assert n <= NPAD

    ei = np.asarray(edge_index, np.int64)
    src = np.concatenate([ei[0], np.arange(n, dtype=np.int64)])
    dst = np.concatenate([ei[1], np.arange(n, dtype=np.int64)])

    deg = np.bincount(dst, minlength=NPAD).astype(np.float32)
    dinv = np.where(deg > 0, 1.0 / np.sqrt(deg), 0.0).astype(np.float32)

    # dense per-block-pair count masks: mask[c, b, s, half, g', j].
    # The src-block axis is permuted into AllGather-section order
    # (dst-sub-block 0-4 | 5-8 | 9) so phase 2A fills table slots in plain
    # slot order as each collective slice lands; slot 79 is the all-padding
    # block (global 79) and is skipped everywhere.
    mask = np.zeros((N_CORES, NBLK, 128, NGB, 128), np.float32)
    c = dst // PER
    b = (dst % PER) // 128
    j = dst % 128
    s = src % 128
    g = src // 128
    np.add.at(mask, (c, b, s, g, j), 1.0)
    mask = mask[:, :, :, GLIST, :].reshape(N_CORES, NBLK, 128, 2, HALF_G, 128)

    W1 = np.asarray(W1, np.float32)
    W2 = np.asarray(W2, np.float32)
    v1 = W2 @ np.asarray(att_src, np.float32)
    v2 = W2 @ np.asarray(att_dst, np.float32)

    Xp = np.zeros((NPAD, D), np.float32)
    Xp[:n] = X
    xt = _bf16(Xp.T.reshape(2, 128, NPAD))

    W2p = np.zeros((D, TW), np.float32)
    W2p[:, :D] = W2
    W2p[:, 256] = v1

    dinv_cols = dinv.reshape(NGB, 128).T          # [128, 80] global order
    dinva = np.ascontiguousarray(dinv_cols[:, GLIST])  # slot order (src side)

    shared = dict(
        xt=xt,
        w1=_bf16(W1.reshape(2, 128, D)),
        w2p=_bf16(W2p.reshape(2, 128, TW)),
        v2c=_bf16(v2.reshape(2, 128, 1)),
        b1r=np.ascontiguousarray(np.tile(np.asarray(b1, np.float32)[None], (128, 1))),
        b2r=np.ascontiguousarray(np.tile(np.asarray(b2, np.float32)[None], (128, 1))),
        dinva=dinva,
        ident=_bf16(np.eye(128, dtype=np.float32)),
        ones1=_bf16(np.ones((1, 128), np.float32)),
    )
    per_core = [
        dict(
            mask=_bf16(mask[cc]),
            maskf8=np.ascontiguousarray(
                mask[cc].astype(ml_dtypes.float8_e4m3)
            ),
            dinvl=np.ascontiguousarray(dinv_cols[:, cc * NBLK:(cc + 1) * NBLK]),
        )
        for cc in range(N_CORES)
    ]
    return shared, per_core, n


# ----------------------------------------------------------------------------
# device program
# ----------------------------------------------------------------------------

def _build_nc(use_collective=True):
    nc = bacc.Bacc(
        "TRN2", target_bir_lowering=False, debug=False, num_devices=N_CORES
    )

    xt_d = nc.dram_tensor("xt", [2, 128, NPAD], BF16, kind="ExternalInput")
    w1_d = nc.dram_tensor("w1", [2, 128, D], BF16, kind="ExternalInput")
    w2p_d = nc.dram_tensor("w2p", [2, 128, TW], BF16, kind="ExternalInput")
    v2c_d = nc.dram_tensor("v2c", [2, 128, 1], BF16, kind="ExternalInput")
    b1r_d = nc.dram_tensor("b1r", [128, D], FP32, kind="ExternalInput")
    b2r_d = nc.dram_tensor("b2r", [128, D], FP32, kind="ExternalInput")
    dinva_d = nc.dram_tensor("dinva", [128, NGB], FP32, kind="ExternalInput")
    ident_d = nc.dram_tensor("ident", [128, 128], BF16, kind="ExternalInput")
    ones1_d = nc.dram_tensor("ones1", [1, 128], BF16, kind="ExternalInput")
    mask_d = nc.dram_tensor(
        "mask", [NBLK, 128, 2, HALF_G, 128], BF16, kind="ExternalInput"
    )
    maskf8_d = nc.dram_tensor(
        "maskf8", [NBLK, 128, 2, HALF_G, 128], FP8, kind="ExternalInput"
    )
    dinvl_d = nc.dram_tensor("dinvl", [128, NBLK], FP32, kind="ExternalInput")
    out_d = nc.dram_tensor("out_slice", [PER, D], FP32, kind="ExternalOutput")

    # three staggered AllGather slices: blocks 0-4 | 5-8 | 9 -- the earlier
    # (larger) ones overlap layer-1 compute, the last is tiny
    AG_SPLITS = AG_BLOCKS
    ht_slice_s, ht_full_s = [], []
    for si, (b0, b1) in enumerate(AG_SPLITS):
        w = (b1 - b0) * 128
        ht_slice_s.append(
            nc.dram_tensor(f"ht_slice_{si}", [2, 128, w], BF16)
        )
        ht_full_s.append(
            nc.dram_tensor(
                f"ht_full_{si}", [N_CORES, 2, 128, w], BF16,
                addr_space="Shared",
            )
        )

    mu, ad, mx = mybir.AluOpType.mult, mybir.AluOpType.add, mybir.AluOpType.max

    with tile.TileContext(nc) as tc:
        with tc.tile_pool(name="const", bufs=1) as cp:
            # w1 on the sync ring (needed first, with xt); the rest on the
            # scalar ring so they don't delay the xt stream
            w1_sb = cp.tile([128, 2, D], BF16)
            for k in range(2):
                nc.sync.dma_start(w1_sb[:, k, :], w1_d[k])
            dinva_sb = cp.tile([128, NGB], FP32)
            nc.scalar.dma_start(dinva_sb[:], dinva_d[:, :])
            ident_sb = cp.tile([128, 128], BF16)
            nc.scalar.dma_start(ident_sb[:], ident_d[:, :])
            ones1_sb = cp.tile([1, 128], BF16)
            nc.scalar.dma_start(ones1_sb[:], ones1_d[:, :])
            b1_sb = cp.tile([128, D], FP32)
            nc.scalar.dma_start(b1_sb[:], b1r_d[:, :])
            b2_sb = cp.tile([128, D], FP32)
            nc.scalar.dma_start(b2_sb[:], b2r_d[:, :])
            dinvl_sb = cp.tile([128, NBLK], FP32)
            nc.scalar.dma_start(dinvl_sb[:], dinvl_d[:, :])
            w2_sb = cp.tile([128, 2, TW], BF16)
            v2_sb = cp.tile([128, 2, 1], BF16)
            for k in range(2):
                nc.scalar.dma_start(w2_sb[:, k, :], w2p_d[k])
                nc.scalar.dma_start(v2_sb[:, k, :], v2c_d[k])

            # tensors that live across phases
            with tc.tile_pool(name="persist", bufs=1) as pper:
                table2_sb = pper.tile([128, NGB, TW], BF16)
                l2bc_sb = pper.tile([128, NBLK, 128], FP16)
                l1grid_sb = pper.tile([128, NGB, 128], FP16)
                l1h_sb = pper.tile([128, NGB], FP16)

                with tc.tile_pool(name="xw1_p", bufs=1) as xwp:
                    xw1_sb = xwp.tile([128, NGB, D], BF16)

                    # ---- phase 1A: table1 = dinv[u] * (X @ W1), SBUF ----
                    half = NPAD // 2
                    with (
                        tc.tile_pool(name="xt_p", bufs=2) as xp,
                        tc.psum_pool(name="ps1_p", bufs=2) as pp1,
                    ):
                        for hh in range(2):
                            xt_sb = xp.tile([128, 2, half], BF16, tag="xt")
                            for k in range(2):
                                nc.sync.dma_start(
                                    xt_sb[:, k, :],
                                    xt_d[k, :, hh * half:(hh + 1) * half],
                                )
                            for jj in range(half // 128):
                                g = hh * (half // 128) + jj
                                if g == NGB - 1:
                                    continue  # all-padding src block
                                slot = POS[g]
                                ps = pp1.tile([128, D], FP32, tag="ps1")
                                for k in range(2):
                                    nc.tensor.matmul(
                                        ps[:],
                                        lhsT=xt_sb[:, k, jj * 128:(jj + 1) * 128],
                                        rhs=w1_sb[:, k, :],
                                        start=(k == 0),
                                        stop=(k == 1),
                                    )
                                # dinv-scaled psum->table copy, alternating
                                # engines so the copy stream keeps up with PE
                                if slot % 2 == 0:
                                    nc.scalar.mul(
                                        xw1_sb[:, slot, :], ps[:],
                                        dinva_sb[:, slot:slot + 1],
                                    )
                                else:
                                    nc.vector.tensor_scalar(
                                        xw1_sb[:, slot, :], ps[:],
                                        dinva_sb[:, slot:slot + 1], None, op0=mu,
                                    )

                    # table2 fixed column 256 -> 1.0 (denominator)
                    nc.vector.memset(table2_sb[:, :, 256:257], 1.0)

                    # ht_sb allocated before the L1 pools so its region is
                    # free early and the post-AllGather loads overlap L1
                    with tc.tile_pool(name="ht2_p", bufs=1) as hp2:
                        ht_sb = hp2.tile([128, 2 * N_CORES, PER], BF16)

                        # ---- phase 1B: GCN aggregate + H1^T + local l2 ----
                        with (
                            tc.tile_pool(name="m1_p", bufs=3) as mp,
                            tc.tile_pool(name="h1_p", bufs=2) as hp,
                            tc.tile_pool(name="ht_p", bufs=1) as htp,
                            tc.tile_pool(name="l2r_p", bufs=2) as lrp,
                            tc.psum_pool(name="psa_p", bufs=2) as ppa,
                            tc.psum_pool(name="pst_p", bufs=2) as ppt,
                        ):
                            ht_st = htp.tile([128, 2, PER], BF16)
                            for b in range(NBLK):
                                psa = ppa.tile([128, D], FP32, tag="agg1")
                                for hh in range(2):
                                    mt = mp.tile(
                                        [128, HALF_G, 128], FP8, tag="m1s"
                                    )
                                    nc.sync.dma_start(mt[:], maskf8_d[b, :, hh])
                                    ng = HALF_G - (1 if hh == 1 else 0)
                                    for gg in range(ng):
                                        g = hh * HALF_G + gg
                                        nc.tensor.matmul(
                                            psa[:],
                                            lhsT=mt[:, gg, :],
                                            rhs=xw1_sb[:, g, :],
                                            start=(g == 0),
                                            stop=(g == NGB - 2),
                                        )
                                h1 = hp.tile([128, D], BF16, tag="h1")
                                nc.vector.scalar_tensor_tensor(
                                    h1[:], psa[:], dinvl_sb[:, b:b + 1], b1_sb[:],
                                    op0=mu, op1=ad,
                                )
                                nc.vector.tensor_scalar_max(h1[:], h1[:], 0.0)
                                for k in range(2):
                                    ptt = ppt.tile([128, 128], BF16, tag="pt")
                                    nc.tensor.transpose(
                                        ptt[:], h1[:, k * 128:(k + 1) * 128],
                                        ident_sb[:],
                                    )
                                    nc.vector.tensor_copy(
                                        ht_st[:, k, b * 128:(b + 1) * 128], ptt[:]
                                    )
                                # local dst logits l2 for this block
                                l2ps = ppt.tile([128, 128], FP32, tag="l2ps")
                                for k in range(2):
                                    nc.tensor.matmul(
                                        l2ps[0:1, :],
                                        lhsT=v2_sb[:, k, :],
                                        rhs=ht_st[:, k, b * 128:(b + 1) * 128],
                                        start=(k == 0),
                                        stop=(k == 1),
                                    )
                                l2row = lrp.tile([1, 128], BF16, tag="l2row")
                                nc.vector.tensor_copy(l2row[:], l2ps[0:1, :])
                                bcps = ppt.tile([128, 128], FP32, tag="bcps")
                                nc.tensor.matmul(
                                    bcps[:], lhsT=ones1_sb[:], rhs=l2row[:],
                                    start=True, stop=True,
                                )
                                nc.vector.tensor_copy(l2bc_sb[:, b, :], bcps[:])
                                # staggered AllGathers fire as soon as their
                                # block range is transposed (scalar DMA ring
                                # jumps the mask-stream queue)
                                for si, (b0, b1) in enumerate(AG_SPLITS):
                                    if b != b1 - 1:
                                        continue
                                    lo, w = b0 * 128, (b1 - b0) * 128
                                    for k in range(2):
                                        nc.scalar.dma_start(
                                            ht_slice_s[si][k],
                                            ht_st[:, k, lo:lo + w],
                                        )
                                    if use_collective:
                                        nc.gpsimd.collective_compute(
                                            "AllGather",
                                            mybir.AluOpType.bypass,
                                            replica_groups=[list(range(N_CORES))],
                                            ins=[ht_slice_s[si][:, :, :]],
                                            outs=[ht_full_s[si][:, :, :, :]],
                                        )
                                    else:
                                        for r in range(N_CORES):
                                            nc.sync.dma_start(
                                                ht_full_s[si][r],
                                                ht_slice_s[si][:, :, :],
                                            )
                                    # slice-a loads ride the scalar ring (it
                                    # is idle mid-L1); later slices load on
                                    # the sync ring so the blocked loads do
                                    # not head-of-line-block the next
                                    # slice's ht write + collective trigger
                                    ld = (
                                        nc.scalar.dma_start if si == 0
                                        else nc.sync.dma_start
                                    )
                                    for r in range(N_CORES):
                                        for k in range(2):
                                            ld(
                                                ht_sb[:, 2 * r + k, lo:lo + w],
                                                ht_full_s[si][r, k],
                                            )

                        # ---- phase 2A: table2 = [H1@W2 | 1], SBUF ----
                        # slots are AllGather-section-ordered by GLIST, so a
                        # plain slot loop consumes each collective slice as
                        # it lands
                        with tc.psum_pool(name="ps3_p", bufs=2) as pp3:
                            for slot in range(NGB - 1):
                                g = GLIST[slot]
                                r, bb = divmod(g, NBLK)
                                ps = pp3.tile([128, TW], FP32, tag="ps3")
                                for k in range(2):
                                    nc.tensor.matmul(
                                        ps[:],
                                        lhsT=ht_sb[
                                            :, 2 * r + k, bb * 128:(bb + 1) * 128
                                        ],
                                        rhs=w2_sb[:, k, :],
                                        start=(k == 0),
                                        stop=(k == 1),
                                    )
                                if slot % 2 == 0:
                                    nc.scalar.copy(
                                        table2_sb[:, slot, 0:256], ps[:, 0:256]
                                    )
                                else:
                                    nc.vector.tensor_copy(
                                        table2_sb[:, slot, 0:256], ps[:, 0:256]
                                    )
                                nc.vector.tensor_copy(
                                    l1h_sb[:, slot:slot + 1], ps[:, 256:257]
                                )

                # materialize l1 source-logit grid (fp16, broadcast over j)
                for hh in range(2):
                    ng = HALF_G - (1 if hh == 1 else 0)
                    nc.vector.tensor_copy(
                        l1grid_sb[:, hh * HALF_G:hh * HALF_G + ng, :],
                        l1h_sb[:, hh * HALF_G:hh * HALF_G + ng]
                        .unsqueeze(-1)
                        .broadcast_to([128, ng, 128]),
                    )

                # ---- phase 2B: GAT aggregate ----
                # Software-pipelined: engines execute their queues in FIFO
                # order, so producers for chunk i are issued before the
                # consumers of chunk i-1, and block outputs lag one more
                # step -- no engine ever heads its queue with a
                # not-yet-satisfiable dependency.
                with (
                    tc.tile_pool(name="m2m_p", bufs=3) as mp2,
                    tc.tile_pool(name="z_p", bufs=2) as zp,
                    tc.tile_pool(name="e_p", bufs=3) as ep,
                    tc.tile_pool(name="a2_p", bufs=2) as ap2,
                    tc.tile_pool(name="o_p", bufs=2) as op_,
                    tc.tile_pool(name="rc_p", bufs=2) as rcp,
                    tc.psum_pool(name="ps4_p", bufs=3) as pp4,
                ):
                    chunks = [(b, hh) for b in range(NBLK) for hh in range(2)]
                    ps_of = {}
                    pending = []  # produced, not yet consumed
                    done_ps = []  # psum tiles awaiting output emission

                    def produce(b, hh):
                        ng = HALF_G - (1 if hh == 1 else 0)
                        mt = mp2.tile([128, HALF_G, 128], BF16, tag="m2s")
                        nc.sync.dma_start(mt[:], mask_d[b, :, hh])
                        z = zp.tile([128, HALF_G, 128], FP16, tag="z")
                        nc.vector.tensor_tensor(
                            z[:, :ng, :],
                            l1grid_sb[:, hh * HALF_G:hh * HALF_G + ng, :],
                            l2bc_sb[:, b:b + 1, :].broadcast_to([128, ng, 128]),
                            op=ad,
                        )
                        # exp(leakyrelu(z)) == max(e^z, e^(0.2 z)) exactly
                        e1 = ep.tile([128, HALF_G, 128], BF16, tag="e1")
                        nc.scalar.activation(
                            e1[:, :ng, :], z[:, :ng, :],
                            mybir.ActivationFunctionType.Exp,
                        )
                        e2 = ep.tile([128, HALF_G, 128], BF16, tag="e2")
                        nc.scalar.activation(
                            e2[:, :ng, :], z[:, :ng, :],
                            mybir.ActivationFunctionType.Exp, scale=0.2,
                        )
                        return (b, hh, ng, mt, e1, e2)

                    def consume(b, hh, ng, mt, e1, e2):
                        nc.vector.tensor_tensor(
                            e1[:, :ng, :], e1[:, :ng, :], e2[:, :ng, :], op=mx
                        )
                        m2 = ap2.tile([128, HALF_G, 128], BF16, tag="m2")
                        nc.vector.tensor_tensor(
                            m2[:, :ng, :], e1[:, :ng, :], mt[:, :ng, :], op=mu
                        )
                        if hh == 0:
                            ps_of[b] = pp4.tile(
                                [128, TW], FP32, tag="agg2", name=f"agg2_{b}"
                            )
                        ps = ps_of[b]
                        for gg in range(ng):
                            g = hh * HALF_G + gg
                            nc.tensor.matmul(
                                ps[:],
                                lhsT=m2[:, gg, :],
                                rhs=table2_sb[:, g, :],
                                start=(g == 0),
                                stop=(g == NGB - 2),
                            )
                        if hh == 1:
                            done_ps.append((b, ps))

                    def emit_output():
                        b, ps = done_ps.pop(0)
                        rc = rcp.tile([128, 1], FP32, tag="rc")
                        nc.vector.reciprocal(rc[:], ps[:, 256:257])
                        ob = op_.tile([128, D], FP32, tag="ob")
                        nc.vector.scalar_tensor_tensor(
                            ob[:], ps[:, 0:D], rc[:], b2_sb[:], op0=mu, op1=ad,
                        )
                        nc.vector.tensor_scalar_max(ob[:], ob[:], 0.0)
                        nc.sync.dma_start(out_d[b * 128:(b + 1) * 128, :], ob[:])

                    for i, (b, hh) in enumerate(chunks):
                        pending.append(produce(b, hh))
                        if len(pending) > 1:
                            consume(*pending.pop(0))
                        if i >= 3 and (i % 2) == 1 and done_ps:
                            emit_output()
                    consume(*pending.pop(0))
                    while done_ps:
                        emit_output()
    nc.finalize()
    return nc


# ----------------------------------------------------------------------------
# entry point
# ----------------------------------------------------------------------------

_CACHE = {}


def _get_nc():
    if "nc" not in _CACHE:
        _CACHE["nc"] = _build_nc()
    return _CACHE["nc"]


def kernel(event_emb, edge_index, W1, b1, W2, att_src, att_dst, b2,
           _want_results=False, _trace=False):
    shared, per_core, n = _prep(
        event_emb, edge_index, W1, b1, W2, att_src, att_dst, b2
    )
    nc = _get_nc()
    in_maps = [{**shared, **per_core[c]} for c in range(N_CORES)]
    res = run_bass_kernel_spmd(
        nc, in_maps, core_ids=list(range(N_CORES)), trace=_trace
    )
    out = np.concatenate(
        [res.results[c]["out_slice"] for c in range(N_CORES)], axis=0
    )[:n]
    if _want_results:
        return out, res
    return out
